# revision 13
# baseline (speedup 1.0000x reference)
"""Trainium2 Bass kernel for nn_Alignment_vector (sparse_attention).

Reference computation per batch b (B=128, Lq=128, Ls=256, d=1024, K=256):
  q = query * matrix                                  (Lq, d)
  A = context @ q.T                                   (Ls, Lq)
  A = leaky_relu(A, 0.1); A = A / ||A||_rows(q-axis)
  attn = softmax(smooth * A.T, axis=s)                (Lq, Ls)
  wc = attn @ context; wc = wc / ||wc||_rows(d-axis)  (Lq, d)
  sim = (query - wc)^2 @ W.T + b; out = sim / ||sim||_rows

Device mapping (per core, 16 batches):
  - A computed in [s, q] layout (lhsT = contextT chunks, rhs = qT chunks);
    leaky-relu + row-l2norm + exp(smooth * a) all free-dim native on ACT/DVE.
  - softmax denominator is skipped: it cancels against the wc row-l2norm.
  - mm2 computed transposed: wcT[d, q] = sum_s context[s, d] * e[s, q], so both
    operands are in natural layouts (no on-chip transposes anywhere).
  - wc column norms via ones-matmul partition reduction; rsqrt broadcast back
    across partitions with a K=1 matmul.
  - simT[d, q] = (qT - wcT * rs)^2 feeds mm3 directly as lhsT; W bias b is
    added with one extra K=1 accumulating matmul.

Host side only reshapes/transposes/casts inputs into device-friendly layouts
and shards along the batch axis across the 8 cores.
"""

import numpy as np
import ml_dtypes

import concourse.bacc as bacc
import concourse.tile as tile
from concourse import mybir
from concourse.bass_utils import run_bass_kernel_spmd

B, LQ, LS, D, KS = 128, 128, 256, 1024, 256
NCORES = 8
BLOC = B // NCORES  # batches per core
DC = D // 128       # d chunks
F32 = mybir.dt.float32
AF = mybir.ActivationFunctionType
ALU = mybir.AluOpType

# matmul operand dtype (flip to False for an fp32 validation build)
MM_BF16 = True
# Prelu == leaky_relu(0.1) on HW (probe-verified); CoreSim lacks Prelu, so
# sim validation uses the equivalent max(x, 0.1x) composition instead.
USE_PRELU = True

_cache = {}


def _build(smooth: float, mm_bf16: bool, debug_taps: bool = False):
    key = (smooth, mm_bf16, USE_PRELU, debug_taps)
    if key in _cache:
        return _cache[key]

    MMDT = mybir.dt.bfloat16 if mm_bf16 else F32
    nc = bacc.Bacc("TRN2", debug=False)
    if debug_taps:
        dtal = nc.dram_tensor("dtal", (128, 2, LQ), F32, kind="ExternalOutput")
        dte = nc.dram_tensor("dte", (128, 2, LQ), MMDT, kind="ExternalOutput")
        dpsW = nc.dram_tensor("dpsW", (128, DC, LQ), F32, kind="ExternalOutput")
        dtrw = nc.dram_tensor("dtrw", (1, LQ), F32, kind="ExternalOutput")
        dtu = nc.dram_tensor("dtu", (128, DC, LQ), F32, kind="ExternalOutput")
        dpsO = nc.dram_tensor("dpsO", (128, KS), F32, kind="ExternalOutput")

    hq = nc.dram_tensor("hq", (BLOC, 128, DC, LQ), MMDT, kind="ExternalInput")
    hm = nc.dram_tensor("hm", (BLOC, 128, DC, LQ), MMDT, kind="ExternalInput")
    hcT = nc.dram_tensor("hcT", (BLOC, 128, DC, LS), MMDT, kind="ExternalInput")
    hc = nc.dram_tensor("hc", (BLOC, 128, 2, D), MMDT, kind="ExternalInput")
    hw = nc.dram_tensor("hw", (128, DC, KS), MMDT, kind="ExternalInput")
    hb = nc.dram_tensor("hb", (1, KS), F32, kind="ExternalInput")
    hout = nc.dram_tensor("hout", (BLOC, LQ, KS), F32, kind="ExternalOutput")

    inv_sm2 = 1.0 / (smooth * smooth)

    with tile.TileContext(nc) as tc:
        with (
            tc.tile_pool(name="const", bufs=1) as cpool,
            tc.tile_pool(name="inp", bufs=2) as ipool,
            tc.tile_pool(name="work", bufs=2) as wpool,
            tc.tile_pool(name="ps_a", bufs=2, space="PSUM") as ps_a,
            tc.tile_pool(name="ps_w", bufs=1, space="PSUM") as ps_w,
            tc.tile_pool(name="ps_s", bufs=1, space="PSUM") as ps_s,
        ):
            tW = cpool.tile([128, DC, KS], MMDT)
            nc.sync.dma_start(out=tW, in_=hw[:, :, :])
            tb = cpool.tile([1, KS], F32)
            nc.sync.dma_start(out=tb, in_=hb[:, :])
            tones = cpool.tile([128, 1], MMDT)
            nc.vector.memset(tones, 1.0)
            tones1 = cpool.tile([1, 128], F32)
            nc.vector.memset(tones1, 1.0)

            for bi in range(BLOC):
                # ---- load inputs of batch bi
                tq = ipool.tile([128, DC, LQ], MMDT, tag="tq")
                tm = ipool.tile([128, DC, LQ], MMDT, tag="tm")
                tcT = ipool.tile([128, DC, LS], MMDT, tag="tcT")
                tcn = ipool.tile([128, 2, D], MMDT, tag="tcn")
                nc.sync.dma_start(out=tq, in_=hq[bi])
                nc.sync.dma_start(out=tm, in_=hm[bi])
                nc.sync.dma_start(out=tcT, in_=hcT[bi])
                nc.sync.dma_start(out=tcn, in_=hc[bi])

                # ---- qT = queryT * matrixT  [d-part, j, q]
                tqT = wpool.tile([128, DC, LQ], MMDT, tag="tqT")
                nc.vector.tensor_mul(tqT, tq, tm)

                # ---- mm1: A[s, q] = sum_d context[s, d] q[q, d]
                psA = ps_a.tile([128, 2, LQ], F32, tag="psA")
                for i in range(2):
                    for j in range(DC):
                        nc.tensor.matmul(
                            psA[:, i, :],
                            tcT[:, j, 128 * i : 128 * i + 128],
                            tqT[:, j, :],
                            start=(j == 0),
                            stop=(j == DC - 1),
                        )

                # ---- leaky relu, row l2norm, exp(smooth * normalized)
                tal = wpool.tile([128, 2, LQ], F32, tag="tal")
                tal01 = None
                if not USE_PRELU:
                    tal01 = wpool.tile([128, 2, LQ], F32, tag="tal01")
                tscrA = wpool.tile([128, 2, LQ], F32, tag="tscrA")
                tn2A = wpool.tile([128, 2], F32, tag="tn2A")
                tnrA = wpool.tile([128, 2], F32, tag="tnrA")
                trs9 = wpool.tile([128, 2], F32, tag="trs9")
                te = wpool.tile([128, 2, LQ], MMDT, tag="te")
                for i in range(2):
                    if USE_PRELU:
                        nc.scalar.activation(
                            tal[:, i, :], psA[:, i, :], AF.Prelu, alpha=0.1
                        )
                    else:
                        # leaky_relu(x, 0.1) = max(x, 0.1 * x)
                        nc.scalar.mul(tal01[:, i, :], psA[:, i, :], 0.1)
                        nc.vector.tensor_max(
                            tal[:, i, :], psA[:, i, :], tal01[:, i, :]
                        )
                    nc.scalar.activation(
                        tscrA[:, i, :],
                        tal[:, i, :],
                        AF.Square,
                        accum_out=tn2A[:, i : i + 1],
                    )
                    # ||a_row|| / smooth, then reciprocal -> smooth / ||a_row||
                    nc.scalar.activation(
                        tnrA[:, i : i + 1], tn2A[:, i : i + 1], AF.Sqrt, scale=inv_sm2
                    )
                    nc.vector.reciprocal(trs9[:, i : i + 1], tnrA[:, i : i + 1])
                    nc.scalar.activation(
                        te[:, i, :], tal[:, i, :], AF.Exp, scale=trs9[:, i : i + 1]
                    )

                # ---- mm2 (transposed): wcT[d, q] = sum_s context[s, d] e[s, q]
                psW = ps_w.tile([128, DC, LQ], F32, tag="psW")
                for j in range(DC):
                    for i in range(2):
                        nc.tensor.matmul(
                            psW[:, j, :],
                            tcn[:, i, 128 * j : 128 * j + 128],
                            te[:, i, :],
                            start=(i == 0),
                            stop=(i == 1),
                        )

                # ---- column l2norm of wcT over d (partition reduction)
                tsq = wpool.tile([128, DC, LQ], MMDT, tag="tsq")
                for j in range(DC):
                    nc.scalar.activation(tsq[:, j, :], psW[:, j, :], AF.Square)
                psN = ps_s.tile([1, LQ], F32, tag="psN")
                for j in range(DC):
                    nc.tensor.matmul(
                        psN, tones, tsq[:, j, :], start=(j == 0), stop=(j == DC - 1)
                    )
                tnw = wpool.tile([1, LQ], F32, tag="tnw")
                trw = wpool.tile([1, LQ], F32, tag="trw")
                nc.scalar.activation(tnw, psN, AF.Sqrt)
                nc.vector.reciprocal(trw, tnw)
                # broadcast rs across partitions with K=1 matmul
                psB = ps_s.tile([128, 128], F32, tag="psB")
                nc.tensor.matmul(psB, tones1, trw, start=True, stop=True)
                trwb = wpool.tile([128, 128], F32, tag="trwb")
                nc.scalar.copy(trwb, psB)

                # ---- simT[d, q] = (qT - wcT * rs)^2
                ttt = wpool.tile([128, DC, LQ], F32, tag="ttt")
                tu = wpool.tile([128, DC, LQ], F32, tag="tu")
                tsim = wpool.tile([128, DC, LQ], MMDT, tag="tsim")
                for j in range(DC):
                    nc.vector.tensor_mul(ttt[:, j, :], psW[:, j, :], trwb)
                    # note: raw query here, not query*matrix
                    nc.vector.tensor_sub(tu[:, j, :], tq[:, j, :], ttt[:, j, :])
                    nc.scalar.activation(tsim[:, j, :], tu[:, j, :], AF.Square)

                # ---- mm3: out[q, k] = sum_d sim[q, d] W[k, d]  (+ bias b)
                psO = ps_s.tile([128, KS], F32, tag="psO")
                for j in range(DC):
                    nc.tensor.matmul(
                        psO, tsim[:, j, :], tW[:, j, :], start=(j == 0), stop=False
                    )
                nc.tensor.matmul(psO, tones1, tb, start=False, stop=True)

                # ---- final row l2norm
                tscrF = wpool.tile([128, KS], F32, tag="tscrF")
                tn2f = wpool.tile([128, 1], F32, tag="tn2f")
                nc.scalar.activation(tscrF, psO, AF.Square, accum_out=tn2f)
                tnf = wpool.tile([128, 1], F32, tag="tnf")
                trf = wpool.tile([128, 1], F32, tag="trf")
                nc.scalar.activation(tnf, tn2f, AF.Sqrt)
                nc.vector.reciprocal(trf, tnf)
                tout = wpool.tile([128, KS], F32, tag="tout")
                nc.vector.tensor_scalar_mul(tout, psO, trf[:, 0:1])
                nc.sync.dma_start(out=hout[bi], in_=tout)

                if debug_taps and bi == 0:
                    nc.sync.dma_start(out=dtal[:, :, :], in_=tal)
                    nc.sync.dma_start(out=dte[:, :, :], in_=te)
                    dbgW = wpool.tile([128, DC, LQ], F32, tag="dbgW")
                    for j in range(DC):
                        nc.scalar.copy(dbgW[:, j, :], psW[:, j, :])
                    nc.sync.dma_start(out=dpsW[:, :, :], in_=dbgW)
                    nc.sync.dma_start(out=dtrw[:, :], in_=trw)
                    nc.sync.dma_start(out=dtu[:, :, :], in_=tu)
                    dbgO = wpool.tile([128, KS], F32, tag="dbgO")
                    nc.scalar.copy(dbgO, psO)
                    nc.sync.dma_start(out=dpsO[:, :], in_=dbgO)

    nc.compile()
    _cache[key] = nc
    return nc


def _prep(query, context, matrix, W, b, mm_bf16):
    npdt = ml_dtypes.bfloat16 if mm_bf16 else np.float32
    # [b, p, j, q] = query[b, q, 128j+p]
    hq = np.ascontiguousarray(
        query.reshape(B, LQ, DC, 128).transpose(0, 3, 2, 1)
    ).astype(npdt)
    hm = np.ascontiguousarray(
        matrix.reshape(B, LQ, DC, 128).transpose(0, 3, 2, 1)
    ).astype(npdt)
    # [b, p, j, s] = context[b, s, 128j+p]
    hcT = np.ascontiguousarray(
        context.reshape(B, LS, DC, 128).transpose(0, 3, 2, 1)
    ).astype(npdt)
    # [b, p, i, d] = context[b, 128i+p, d]
    hc = np.ascontiguousarray(
        context.reshape(B, 2, 128, D).transpose(0, 2, 1, 3)
    ).astype(npdt)
    # [p, j, k] = W[k, 128j+p]
    hw = np.ascontiguousarray(W.reshape(KS, DC, 128).transpose(2, 1, 0)).astype(npdt)
    hb = np.ascontiguousarray(b.reshape(1, KS)).astype(np.float32)
    return hq, hm, hcT, hc, hw, hb


def kernel(query, context, matrix, W, b, smooth, _trace=False):
    query = np.asarray(query, dtype=np.float32)
    context = np.asarray(context, dtype=np.float32)
    matrix = np.asarray(matrix, dtype=np.float32)
    W = np.asarray(W, dtype=np.float32)
    b = np.asarray(b, dtype=np.float32)

    nc = _build(float(smooth), MM_BF16)
    hq, hm, hcT, hc, hw, hb = _prep(query, context, matrix, W, b, MM_BF16)

    in_maps = []
    for c in range(NCORES):
        sl = slice(c * BLOC, (c + 1) * BLOC)
        in_maps.append(
            {
                "hq": hq[sl],
                "hm": hm[sl],
                "hcT": hcT[sl],
                "hc": hc[sl],
                "hw": hw,
                "hb": hb,
            }
        )

    res = run_bass_kernel_spmd(
        nc, in_maps, core_ids=list(range(NCORES)), trace=_trace
    )
    out = np.concatenate([r["hout"] for r in res.results], axis=0)
    out = np.ascontiguousarray(out.astype(np.float32))
    if _trace:
        return out, res
    return out


# revision 16
# speedup vs baseline: 1.0871x; 1.0871x over previous
"""Trainium2 Bass kernel for nn_Alignment_vector (sparse_attention).

Reference computation per batch b (B=128, Lq=128, Ls=256, d=1024, K=256):
  q = query * matrix                                  (Lq, d)
  A = context @ q.T                                   (Ls, Lq)
  A = leaky_relu(A, 0.1); A = A / ||A||_rows(q-axis)
  attn = softmax(smooth * A.T, axis=s)                (Lq, Ls)
  wc = attn @ context; wc = wc / ||wc||_rows(d-axis)  (Lq, d)
  sim = (query - wc)^2 @ W.T + b; out = sim / ||sim||_rows

Device mapping (per core, 16 batches):
  - A computed in [s, q] layout (lhsT = contextT chunks, rhs = qT chunks);
    leaky-relu + row-l2norm + exp(smooth * a) all free-dim native on ACT/DVE.
  - softmax denominator is skipped: it cancels against the wc row-l2norm.
  - mm2 computed transposed: wcT[d, q] = sum_s context[s, d] * e[s, q], so both
    operands are in natural layouts (no on-chip transposes anywhere).
  - wc column norms via ones-matmul partition reduction; rsqrt broadcast back
    across partitions with a K=1 matmul.
  - simT[d, q] = (qT - wcT * rs)^2 feeds mm3 directly as lhsT; W bias b is
    added with one extra K=1 accumulating matmul.

Host side only reshapes/transposes/casts inputs into device-friendly layouts
and shards along the batch axis across the 8 cores.
"""

import numpy as np
import ml_dtypes

import concourse.bass as bass
import concourse.bacc as bacc
import concourse.tile as tile
from concourse import mybir
from concourse.bass_utils import run_bass_kernel_spmd

B, LQ, LS, D, KS = 128, 128, 256, 1024, 256
NCORES = 8
BLOC = B // NCORES  # batches per core
DC = D // 128       # d chunks
F32 = mybir.dt.float32
AF = mybir.ActivationFunctionType
ALU = mybir.AluOpType

# matmul operand dtype (flip to False for an fp32 validation build)
MM_BF16 = True
# Prelu == leaky_relu(0.1) on HW (probe-verified); CoreSim lacks Prelu, so
# sim validation uses the equivalent max(x, 0.1x) composition instead.
USE_PRELU = True

_cache = {}


def _build(smooth: float, mm_bf16: bool, debug_taps: bool = False):
    key = (smooth, mm_bf16, USE_PRELU, debug_taps)
    if key in _cache:
        return _cache[key]

    MMDT = mybir.dt.bfloat16 if mm_bf16 else F32
    nc = bacc.Bacc("TRN2", debug=False)
    if debug_taps:
        dtal = nc.dram_tensor("dtal", (128, 2, LQ), F32, kind="ExternalOutput")
        dte = nc.dram_tensor("dte", (128, 2, LQ), MMDT, kind="ExternalOutput")
        dpsW = nc.dram_tensor("dpsW", (128, DC, LQ), F32, kind="ExternalOutput")
        dtrw = nc.dram_tensor("dtrw", (1, LQ), F32, kind="ExternalOutput")
        dtu = nc.dram_tensor("dtu", (128, DC, LQ), F32, kind="ExternalOutput")
        dpsO = nc.dram_tensor("dpsO", (128, KS), F32, kind="ExternalOutput")

    hq = nc.dram_tensor("hq", (BLOC, 128, DC, LQ), MMDT, kind="ExternalInput")
    hm = nc.dram_tensor("hm", (BLOC, 128, DC, LQ), MMDT, kind="ExternalInput")
    hcT = nc.dram_tensor("hcT", (BLOC, 128, DC, LS), MMDT, kind="ExternalInput")
    hc = nc.dram_tensor("hc", (BLOC, 128, 2, D), MMDT, kind="ExternalInput")
    hw = nc.dram_tensor("hw", (128, DC, KS), MMDT, kind="ExternalInput")
    hb = nc.dram_tensor("hb", (1, KS), F32, kind="ExternalInput")
    hout = nc.dram_tensor("hout", (BLOC, LQ, KS), F32, kind="ExternalOutput")

    inv_sm2 = 1.0 / (smooth * smooth)

    with tile.TileContext(nc) as tc:
        with (
            tc.tile_pool(name="const", bufs=1) as cpool,
            tc.tile_pool(name="inp", bufs=2) as ipool,
            tc.tile_pool(name="work", bufs=2) as wpool,
            tc.tile_pool(name="ps_a", bufs=2, space="PSUM") as ps_a,
            tc.tile_pool(name="ps_w", bufs=1, space="PSUM") as ps_w,
            tc.tile_pool(name="ps_s", bufs=1, space="PSUM") as ps_s,
        ):
            tW = cpool.tile([128, DC, KS], MMDT)
            nc.sync.dma_start(out=tW, in_=hw[:, :, :])
            tb = cpool.tile([1, KS], F32)
            nc.sync.dma_start(out=tb, in_=hb[:, :])
            tones = cpool.tile([128, 1], MMDT)
            nc.vector.memset(tones, 1.0)
            tones1 = cpool.tile([1, 128], F32)
            nc.vector.memset(tones1, 1.0)

            for bi in range(BLOC):
                # ---- load inputs of batch bi
                tq = ipool.tile([128, DC, LQ], MMDT, tag="tq")
                tm = ipool.tile([128, DC, LQ], MMDT, tag="tm")
                tcT = ipool.tile([128, DC, LS], MMDT, tag="tcT")
                tcn = ipool.tile([128, 2, D], MMDT, tag="tcn")
                nc.sync.dma_start(out=tq, in_=hq[bi])
                nc.sync.dma_start(out=tm, in_=hm[bi])
                nc.sync.dma_start(out=tcT, in_=hcT[bi])
                nc.sync.dma_start(out=tcn, in_=hc[bi])

                # ---- qT = queryT * matrixT  [d-part, j, q]
                tqT = wpool.tile([128, DC, LQ], MMDT, tag="tqT")
                nc.vector.tensor_mul(tqT, tq, tm)

                # ---- mm1: A[s, q] = sum_d context[s, d] q[q, d]
                psA = ps_a.tile([128, 2, LQ], F32, tag="psA")
                for i in range(2):
                    for j in range(DC):
                        nc.tensor.matmul(
                            psA[:, i, :],
                            tcT[:, j, 128 * i : 128 * i + 128],
                            tqT[:, j, :],
                            start=(j == 0),
                            stop=(j == DC - 1),
                        )

                # ---- leaky relu, row l2norm, exp(smooth * normalized)
                tal = wpool.tile([128, 2, LQ], F32, tag="tal")
                tal01 = None
                if not USE_PRELU:
                    tal01 = wpool.tile([128, 2, LQ], F32, tag="tal01")
                tsqA = wpool.tile([128, 2, LQ], F32, tag="tsqA")
                tn2A = wpool.tile([128, 2], F32, tag="tn2A")
                tnrA = wpool.tile([128, 2], F32, tag="tnrA")
                trs9 = wpool.tile([128, 2], F32, tag="trs9")
                te = wpool.tile([128, 2, LQ], MMDT, tag="te")
                if USE_PRELU:
                    nc.scalar.activation(
                        tal.rearrange("p a q -> p (a q)"),
                        psA.rearrange("p a q -> p (a q)"),
                        AF.Prelu,
                        alpha=0.1,
                    )
                else:
                    # leaky_relu(x, 0.1) = max(x, 0.1 * x)
                    nc.scalar.mul(tal01, psA, 0.1)
                    nc.vector.tensor_max(tal, psA, tal01)
                nc.gpsimd.tensor_mul(tsqA, tal, tal)
                for i in range(2):
                    nc.vector.reduce_sum(
                        tn2A[:, i : i + 1], tsqA[:, i, :], axis=mybir.AxisListType.X
                    )
                # ||a_row|| / smooth, then reciprocal -> smooth / ||a_row||
                nc.scalar.activation(tnrA, tn2A, AF.Sqrt, scale=inv_sm2)
                nc.vector.reciprocal(trs9, tnrA)
                for i in range(2):
                    nc.scalar.activation(
                        te[:, i, :], tal[:, i, :], AF.Exp, scale=trs9[:, i : i + 1]
                    )

                # ---- mm2 (transposed): wcT[d, q] = sum_s context[s, d] e[s, q]
                psW = ps_w.tile([128, DC, LQ], F32, tag="psW")
                for j in range(DC):
                    for i in range(2):
                        nc.tensor.matmul(
                            psW[:, j, :],
                            tcn[:, i, 128 * j : 128 * j + 128],
                            te[:, i, :],
                            start=(i == 0),
                            stop=(i == 1),
                        )

                # ---- move wcT to SBUF, column l2norm over d (partition reduction)
                tw = wpool.tile([128, DC, LQ], F32, tag="tw")
                nc.vector.tensor_copy(tw, psW)
                tsq = wpool.tile([128, DC, LQ], MMDT, tag="tsq")
                nc.gpsimd.tensor_mul(tsq, tw, tw)
                psN = ps_s.tile([1, LQ], F32, tag="psN")
                for j in range(DC):
                    nc.tensor.matmul(
                        psN, tones, tsq[:, j, :], start=(j == 0), stop=(j == DC - 1)
                    )
                tnw = wpool.tile([1, LQ], F32, tag="tnw")
                trw = wpool.tile([1, LQ], F32, tag="trw")
                nc.scalar.activation(tnw, psN, AF.Sqrt)
                nc.vector.reciprocal(trw, tnw)
                # broadcast rs across partitions with K=1 matmul
                psB = ps_s.tile([128, 128], F32, tag="psB")
                nc.tensor.matmul(psB, tones1, trw, start=True, stop=True)
                trwb = wpool.tile([128, 128], F32, tag="trwb")
                nc.vector.tensor_copy(trwb, psB)

                # ---- simT[d, q] = (qT - wcT * rs)^2
                trwb_b = bass.AP(
                    tensor=trwb.tensor,
                    offset=trwb.offset,
                    ap=[list(trwb.ap[0]), [0, DC], list(trwb.ap[1])],
                )
                ttt = wpool.tile([128, DC, LQ], F32, tag="ttt")
                tu = wpool.tile([128, DC, LQ], F32, tag="tu")
                tsim = wpool.tile([128, DC, LQ], MMDT, tag="tsim")
                nc.vector.tensor_mul(ttt, tw, trwb_b)
                # note: raw query here, not query*matrix
                nc.vector.tensor_sub(tu, tq, ttt)
                nc.gpsimd.tensor_mul(tsim, tu, tu)

                # ---- mm3: out[q, k] = sum_d sim[q, d] W[k, d]  (+ bias b)
                psO = ps_s.tile([128, KS], F32, tag="psO")
                for j in range(DC):
                    nc.tensor.matmul(
                        psO, tsim[:, j, :], tW[:, j, :], start=(j == 0), stop=False
                    )
                nc.tensor.matmul(psO, tones1, tb, start=False, stop=True)

                # ---- final row l2norm
                tscrF = wpool.tile([128, KS], F32, tag="tscrF")
                tn2f = wpool.tile([128, 1], F32, tag="tn2f")
                nc.scalar.activation(tscrF, psO, AF.Square, accum_out=tn2f)
                tnf = wpool.tile([128, 1], F32, tag="tnf")
                trf = wpool.tile([128, 1], F32, tag="trf")
                nc.scalar.activation(tnf, tn2f, AF.Sqrt)
                nc.vector.reciprocal(trf, tnf)
                tout = wpool.tile([128, KS], F32, tag="tout")
                nc.vector.tensor_scalar_mul(tout, psO, trf[:, 0:1])
                nc.sync.dma_start(out=hout[bi], in_=tout)

                if debug_taps and bi == 0:
                    nc.sync.dma_start(out=dtal[:, :, :], in_=tal)
                    nc.sync.dma_start(out=dte[:, :, :], in_=te)
                    dbgW = wpool.tile([128, DC, LQ], F32, tag="dbgW")
                    for j in range(DC):
                        nc.scalar.copy(dbgW[:, j, :], psW[:, j, :])
                    nc.sync.dma_start(out=dpsW[:, :, :], in_=dbgW)
                    nc.sync.dma_start(out=dtrw[:, :], in_=trw)
                    nc.sync.dma_start(out=dtu[:, :, :], in_=tu)
                    dbgO = wpool.tile([128, KS], F32, tag="dbgO")
                    nc.scalar.copy(dbgO, psO)
                    nc.sync.dma_start(out=dpsO[:, :], in_=dbgO)

    nc.compile()
    _cache[key] = nc
    return nc


def _prep(query, context, matrix, W, b, mm_bf16):
    npdt = ml_dtypes.bfloat16 if mm_bf16 else np.float32
    # [b, p, j, q] = query[b, q, 128j+p]
    hq = np.ascontiguousarray(
        query.reshape(B, LQ, DC, 128).transpose(0, 3, 2, 1)
    ).astype(npdt)
    hm = np.ascontiguousarray(
        matrix.reshape(B, LQ, DC, 128).transpose(0, 3, 2, 1)
    ).astype(npdt)
    # [b, p, j, s] = context[b, s, 128j+p]
    hcT = np.ascontiguousarray(
        context.reshape(B, LS, DC, 128).transpose(0, 3, 2, 1)
    ).astype(npdt)
    # [b, p, i, d] = context[b, 128i+p, d]
    hc = np.ascontiguousarray(
        context.reshape(B, 2, 128, D).transpose(0, 2, 1, 3)
    ).astype(npdt)
    # [p, j, k] = W[k, 128j+p]
    hw = np.ascontiguousarray(W.reshape(KS, DC, 128).transpose(2, 1, 0)).astype(npdt)
    hb = np.ascontiguousarray(b.reshape(1, KS)).astype(np.float32)
    return hq, hm, hcT, hc, hw, hb


def kernel(query, context, matrix, W, b, smooth, _trace=False):
    query = np.asarray(query, dtype=np.float32)
    context = np.asarray(context, dtype=np.float32)
    matrix = np.asarray(matrix, dtype=np.float32)
    W = np.asarray(W, dtype=np.float32)
    b = np.asarray(b, dtype=np.float32)

    nc = _build(float(smooth), MM_BF16)
    hq, hm, hcT, hc, hw, hb = _prep(query, context, matrix, W, b, MM_BF16)

    in_maps = []
    for c in range(NCORES):
        sl = slice(c * BLOC, (c + 1) * BLOC)
        in_maps.append(
            {
                "hq": hq[sl],
                "hm": hm[sl],
                "hcT": hcT[sl],
                "hc": hc[sl],
                "hw": hw,
                "hb": hb,
            }
        )

    res = run_bass_kernel_spmd(
        nc, in_maps, core_ids=list(range(NCORES)), trace=_trace
    )
    out = np.concatenate([r["hout"] for r in res.results], axis=0)
    out = np.ascontiguousarray(out.astype(np.float32))
    if _trace:
        return out, res
    return out


# revision 24
# speedup vs baseline: 1.5309x; 1.4083x over previous
"""Trainium2 Bass kernel for nn_Alignment_vector (sparse_attention).

Reference computation per batch b (B=128, Lq=128, Ls=256, d=1024, K=256):
  q = query * matrix                                  (Lq, d)
  A = context @ q.T                                   (Ls, Lq)
  A = leaky_relu(A, 0.1); A = A / ||A||_rows(q-axis)
  attn = softmax(smooth * A.T, axis=s)                (Lq, Ls)
  wc = attn @ context; wc = wc / ||wc||_rows(d-axis)  (Lq, d)
  sim = (query - wc)^2 @ W.T + b; out = sim / ||sim||_rows

Device mapping (per core, 16 batches):
  - A computed in [s, q] layout (lhsT = contextT chunks, rhs = qT chunks);
    leaky-relu + row-l2norm + exp(smooth * a) all free-dim native on ACT/DVE.
  - softmax denominator is skipped: it cancels against the wc row-l2norm.
  - mm2 computed transposed: wcT[d, q] = sum_s context[s, d] * e[s, q], so both
    operands are in natural layouts (no on-chip transposes anywhere).
  - wc column norms via ones-matmul partition reduction; rsqrt broadcast back
    across partitions with a K=1 matmul.
  - simT[d, q] = (qT - wcT * rs)^2 feeds mm3 directly as lhsT; W bias b is
    added with one extra K=1 accumulating matmul.

Host side only reshapes/transposes/casts inputs into device-friendly layouts
and shards along the batch axis across the 8 cores.
"""

import numpy as np
import ml_dtypes

import concourse.bass as bass
import concourse.bacc as bacc
import concourse.tile as tile
from concourse import mybir
from concourse.bass_utils import run_bass_kernel_spmd

B, LQ, LS, D, KS = 128, 128, 256, 1024, 256
NCORES = 8
BLOC = B // NCORES  # batches per core
DC = D // 128       # d chunks
F32 = mybir.dt.float32
AF = mybir.ActivationFunctionType
ALU = mybir.AluOpType

# matmul operand dtype (flip to False for an fp32 validation build)
MM_BF16 = True
# Prelu == leaky_relu(0.1) on HW (probe-verified); CoreSim lacks Prelu, so
# sim validation uses the equivalent max(x, 0.1x) composition instead.
USE_PRELU = True

_cache = {}


def _build(smooth: float, mm_bf16: bool, debug_taps: bool = False):
    key = (smooth, mm_bf16, USE_PRELU, debug_taps)
    if key in _cache:
        return _cache[key]

    MMDT = mybir.dt.bfloat16 if mm_bf16 else F32
    nc = bacc.Bacc("TRN2", debug=False)
    if debug_taps:
        dtal = nc.dram_tensor("dtal", (128, 2, LQ), F32, kind="ExternalOutput")
        dte = nc.dram_tensor("dte", (128, 2, LQ), MMDT, kind="ExternalOutput")
        dpsW = nc.dram_tensor("dpsW", (128, DC, LQ), F32, kind="ExternalOutput")
        dtrw = nc.dram_tensor("dtrw", (1, LQ), F32, kind="ExternalOutput")
        dtu = nc.dram_tensor("dtu", (128, DC, LQ), F32, kind="ExternalOutput")
        dpsO = nc.dram_tensor("dpsO", (128, KS), F32, kind="ExternalOutput")

    # hqm packs queryT and matrixT; hctx packs contextT and context-native
    hqm = nc.dram_tensor("hqm", (BLOC, 128, 2, DC, LQ), MMDT, kind="ExternalInput")
    hctx = nc.dram_tensor("hctx", (BLOC, 128, 2, 2048), MMDT, kind="ExternalInput")
    hw = nc.dram_tensor("hw", (128, DC, KS), MMDT, kind="ExternalInput")
    hb = nc.dram_tensor("hb", (1, KS), F32, kind="ExternalInput")
    hout = nc.dram_tensor("hout", (BLOC, LQ, KS), F32, kind="ExternalOutput")

    inv_sm2 = 1.0 / (smooth * smooth)

    with tile.TileContext(nc) as tc:
        with (
            tc.tile_pool(name="const", bufs=1) as cpool,
            tc.tile_pool(name="inp", bufs=3) as ipool,
            tc.tile_pool(name="work", bufs=3) as wpool,
            tc.tile_pool(name="ps_a", bufs=2, space="PSUM") as ps_a,
            tc.tile_pool(name="ps_w", bufs=2, space="PSUM") as ps_w,
            tc.tile_pool(name="ps_s", bufs=2, space="PSUM") as ps_s,
        ):
            tW = cpool.tile([128, DC, KS], MMDT)
            nc.sync.dma_start(out=tW, in_=hw[:, :, :])
            tb = cpool.tile([1, KS], F32)
            nc.sync.dma_start(out=tb, in_=hb[:, :])
            tones = cpool.tile([128, 1], MMDT)
            nc.vector.memset(tones, 1.0)
            tones1 = cpool.tile([1, 128], F32)
            nc.vector.memset(tones1, 1.0)

            for bi in range(BLOC):
                # ---- load inputs of batch bi (2 packed DMAs)
                tqm = ipool.tile([128, 2, DC, LQ], MMDT, tag="tqm")
                tctx = ipool.tile([128, 2, 2048], MMDT, tag="tctx")
                nc.sync.dma_start(out=tqm, in_=hqm[bi])
                nc.sync.dma_start(out=tctx, in_=hctx[bi])
                tq = tqm[:, 0]
                tm = tqm[:, 1]
                tcT = tctx[:, 0].rearrange("p (a s) -> p a s", a=DC)
                tcn = tctx[:, 1].rearrange("p (a d) -> p a d", a=2)

                # ---- qT = queryT * matrixT  [d-part, j, q]
                tqT = wpool.tile([128, DC, LQ], MMDT, tag="tqT")
                nc.vector.tensor_mul(tqT, tq, tm)

                # ---- mm1: A[s, q] = sum_d context[s, d] q[q, d]
                psA = ps_a.tile([128, 2, LQ], F32, tag="psA")
                for i in range(2):
                    for j in range(DC):
                        nc.tensor.matmul(
                            psA[:, i, :],
                            tcT[:, j, 128 * i : 128 * i + 128],
                            tqT[:, j, :],
                            start=(j == 0),
                            stop=(j == DC - 1),
                        )

                # ---- leaky relu, row l2norm, exp(smooth * normalized)
                tal = wpool.tile([128, 2, LQ], F32, tag="tal")
                tal01 = wpool.tile([128, 2, LQ], F32, tag="tal01")
                tsqA = wpool.tile([128, 2, LQ], F32, tag="tsqA")
                tn2A = wpool.tile([128, 2], F32, tag="tn2A")
                tnrA = wpool.tile([128, 2], F32, tag="tnrA")
                trs9 = wpool.tile([128, 2], F32, tag="trs9")
                te = wpool.tile([128, 2, LQ], MMDT, tag="te")
                # leaky_relu(x, 0.1) = max(x, 0.1 * x), on DVE to keep the
                # ACT table set small
                nc.vector.tensor_scalar_mul(tal01, psA, 0.1)
                nc.vector.tensor_max(tal, psA, tal01)
                nc.gpsimd.tensor_mul(tsqA, tal, tal)
                for i in range(2):
                    nc.vector.reduce_sum(
                        tn2A[:, i : i + 1], tsqA[:, i, :], axis=mybir.AxisListType.X
                    )
                # ||a_row|| / smooth, then reciprocal -> smooth / ||a_row||
                nc.scalar.activation(tnrA, tn2A, AF.Sqrt, scale=inv_sm2)
                nc.vector.reciprocal(trs9, tnrA)
                for i in range(2):
                    nc.scalar.activation(
                        te[:, i, :], tal[:, i, :], AF.Exp, scale=trs9[:, i : i + 1]
                    )

                # ---- mm2 (transposed): wcT[d, q] = sum_s context[s, d] e[s, q]
                psW = ps_w.tile([128, DC, LQ], F32, tag="psW")
                for j in range(DC):
                    for i in range(2):
                        nc.tensor.matmul(
                            psW[:, j, :],
                            tcn[:, i, 128 * j : 128 * j + 128],
                            te[:, i, :],
                            start=(i == 0),
                            stop=(i == 1),
                        )

                # ---- column l2norm of wcT over d (partition reduction)
                # psN/psB/psO share one PSUM bank tile [128, 512]
                psS = ps_s.tile([128, 512], F32, tag="psS")
                psN = psS[0:1, 0:LQ]
                psB = psS[:, LQ : 2 * LQ]
                psO = psS[:, 2 * LQ : 2 * LQ + KS]
                tsq = wpool.tile([128, DC, LQ], MMDT, tag="tsq")
                nc.scalar.activation(
                    tsq.rearrange("p a q -> p (a q)"),
                    psW.rearrange("p a q -> p (a q)"),
                    AF.Square,
                )
                for j in range(DC):
                    nc.tensor.matmul(
                        psN, tones, tsq[:, j, :], start=(j == 0), stop=(j == DC - 1)
                    )
                tnw = wpool.tile([1, LQ], F32, tag="tnw")
                trw = wpool.tile([1, LQ], F32, tag="trw")
                nc.scalar.activation(tnw, psN, AF.Sqrt)
                nc.vector.reciprocal(trw, tnw)
                # broadcast rs across partitions with K=1 matmul
                nc.tensor.matmul(psB, tones1, trw, start=True, stop=True)
                trwb = wpool.tile([128, 128], F32, tag="trwb")
                nc.vector.tensor_copy(trwb, psB)

                # ---- simT[d, q] = (qT - wcT * rs)^2
                trwb_b = bass.AP(
                    tensor=trwb.tensor,
                    offset=trwb.offset,
                    ap=[list(trwb.ap[0]), [0, DC], list(trwb.ap[1])],
                )
                ttt = wpool.tile([128, DC, LQ], F32, tag="ttt")
                tu = wpool.tile([128, DC, LQ], F32, tag="tu")
                tsim = wpool.tile([128, DC, LQ], MMDT, tag="tsim")
                nc.vector.tensor_mul(ttt, psW, trwb_b)
                # note: raw query here, not query*matrix
                nc.vector.tensor_sub(tu, tq, ttt)
                nc.gpsimd.tensor_mul(tsim, tu, tu)

                # ---- mm3: out[q, k] = sum_d sim[q, d] W[k, d]  (+ bias b)
                for j in range(DC):
                    nc.tensor.matmul(
                        psO, tsim[:, j, :], tW[:, j, :], start=(j == 0), stop=False
                    )
                nc.tensor.matmul(psO, tones1, tb, start=False, stop=True)

                # ---- final row l2norm
                tscrF = wpool.tile([128, KS], F32, tag="tscrF")
                tn2f = wpool.tile([128, 1], F32, tag="tn2f")
                nc.scalar.activation(tscrF, psO, AF.Square, accum_out=tn2f)
                tnf = wpool.tile([128, 1], F32, tag="tnf")
                trf = wpool.tile([128, 1], F32, tag="trf")
                nc.scalar.activation(tnf, tn2f, AF.Sqrt)
                nc.vector.reciprocal(trf, tnf)
                tout = wpool.tile([128, KS], F32, tag="tout")
                nc.vector.tensor_scalar_mul(tout, psO, trf[:, 0:1])
                nc.sync.dma_start(out=hout[bi], in_=tout)

                if debug_taps and bi == 0:
                    nc.sync.dma_start(out=dtal[:, :, :], in_=tal)
                    nc.sync.dma_start(out=dte[:, :, :], in_=te)
                    dbgW = wpool.tile([128, DC, LQ], F32, tag="dbgW")
                    for j in range(DC):
                        nc.scalar.copy(dbgW[:, j, :], psW[:, j, :])
                    nc.sync.dma_start(out=dpsW[:, :, :], in_=dbgW)
                    nc.sync.dma_start(out=dtrw[:, :], in_=trw)
                    nc.sync.dma_start(out=dtu[:, :, :], in_=tu)
                    dbgO = wpool.tile([128, KS], F32, tag="dbgO")
                    nc.scalar.copy(dbgO, psO)
                    nc.sync.dma_start(out=dpsO[:, :], in_=dbgO)

    nc.compile()
    _cache[key] = nc
    return nc


def _prep(query, context, matrix, W, b, mm_bf16):
    npdt = ml_dtypes.bfloat16 if mm_bf16 else np.float32
    # [b, p, j, q] = query[b, q, 128j+p]
    hq = query.reshape(B, LQ, DC, 128).transpose(0, 3, 2, 1)
    hm = matrix.reshape(B, LQ, DC, 128).transpose(0, 3, 2, 1)
    hqm = np.ascontiguousarray(np.stack([hq, hm], axis=2)).astype(npdt)
    # [b, p, j, s] = context[b, s, 128j+p]
    hcT = context.reshape(B, LS, DC, 128).transpose(0, 3, 2, 1).reshape(B, 128, 2048)
    # [b, p, i, d] = context[b, 128i+p, d]
    hc = context.reshape(B, 2, 128, D).transpose(0, 2, 1, 3).reshape(B, 128, 2048)
    hctx = np.ascontiguousarray(np.stack([hcT, hc], axis=2)).astype(npdt)
    # [p, j, k] = W[k, 128j+p]
    hw = np.ascontiguousarray(W.reshape(KS, DC, 128).transpose(2, 1, 0)).astype(npdt)
    hb = np.ascontiguousarray(b.reshape(1, KS)).astype(np.float32)
    return hqm, hctx, hw, hb


def kernel(query, context, matrix, W, b, smooth, _trace=False):
    query = np.asarray(query, dtype=np.float32)
    context = np.asarray(context, dtype=np.float32)
    matrix = np.asarray(matrix, dtype=np.float32)
    W = np.asarray(W, dtype=np.float32)
    b = np.asarray(b, dtype=np.float32)

    nc = _build(float(smooth), MM_BF16)
    hqm, hctx, hw, hb = _prep(query, context, matrix, W, b, MM_BF16)

    in_maps = []
    for c in range(NCORES):
        sl = slice(c * BLOC, (c + 1) * BLOC)
        in_maps.append(
            {
                "hqm": hqm[sl],
                "hctx": hctx[sl],
                "hw": hw,
                "hb": hb,
            }
        )

    res = run_bass_kernel_spmd(
        nc, in_maps, core_ids=list(range(NCORES)), trace=_trace
    )
    out = np.concatenate([r["hout"] for r in res.results], axis=0)
    out = np.ascontiguousarray(out.astype(np.float32))
    if _trace:
        return out, res
    return out
